# revision 1
# baseline (speedup 1.0000x reference)
"""Trainium2 Bass kernel: MLP-scored masked attention (sparse_attention).

Reference computation per batch b (B=4096, S=200, F=64):
    att_x = concat([q, k, q-k, q*k])            # [S, 256]
    h1 = relu(att_x @ W1 + b1)                  # [S, 80]
    h2 = relu(h1 @ W2 + b2)                     # [S, 40]
    sc = h2 @ W3 + b3                           # [S, 1]
    sc = where(arange(S) < seq_len, sc, NEG_BIG)
    p  = softmax(sc)
    out = p @ k                                 # [1, 64]

Key algebra: with W1 = [W1q; W1k; W1d; W1m] (row blocks of 64),
    att_x @ W1 = q@(W1q+W1d) + k@(W1k-W1d) + (q*k)@W1m
so per batch A_b = q@(W1q+W1d) + b1 is an [80] vector folded into the relu
bias, and the per-(b,s) work is one K=128 matmul with stationary
Ws = [W1k-W1d; W1m] against rhs = [k^T; (q*k)^T].  b3 is softmax-invariant
and dropped.

Distribution: pure data-parallel, batch 4096 sharded over 8 cores (512 each).
Compute dtype bf16 (fp32 softmax/biases); keys are host-converted to bf16 and
s-padded to 208 so the DMA XBAR transpose constraints (p%16, free%128) hold.

Walrus constraint: every Matmult carries at most ONE fresh semaphore wait, so
"observer" matmuls introduce each new semaphore (weight DMAs, identity, the
per-tile probability transposes) to the PE before the real consumers run.
"""

import numpy as np
import os
import sys

sys.path.insert(0, "/opt/trn_rl_repo")

import ml_dtypes
from concourse import bass, mybir, masks
from concourse.tile import TileContext
from concourse.bass_utils import run_bass_kernel_spmd

BF16 = mybir.dt.bfloat16
F32 = mybir.dt.float32

PROBE = os.environ.get("KPROBE", "")
B, S, F = 4096, 200, 64
S2 = 208            # s padded to a multiple of 16 for the XBAR transpose
H1, H2 = 80, 40
NCORES = 8
BPC = B // NCORES   # 512 batches per core
TILE = 64           # batches per tile
NT = BPC // TILE    # 8 tiles
PAIRS = TILE // 2   # 32 pairs per tile
NEG_BIG = float(-(2**32) + 1)
SPLIT_WAITS = True  # hoist multi-waits for walrus; CoreSim can't execute bare drains


def build_graph():
    nc = bass.Bass()

    keys_e = nc.declare_dram_parameter("keys", [BPC, S2, F], BF16, isOutput=False)
    keysT_e = nc.declare_dram_parameter(
        "keysT", [BPC // 2, 2 * F, S2], BF16, isOutput=False
    )
    qT_e = nc.declare_dram_parameter("qT", [F, BPC], F32, isOutput=False)
    seqf_e = nc.declare_dram_parameter("seqf", [BPC, 1], F32, isOutput=False)
    Ws_e = nc.declare_dram_parameter("Ws", [128, H1], BF16, isOutput=False)
    Wqd_e = nc.declare_dram_parameter("Wqd", [F, H1], F32, isOutput=False)
    W2p_e = nc.declare_dram_parameter("W2p", [H1, 64], BF16, isOutput=False)
    W3pp_e = nc.declare_dram_parameter("W3pp", [128, 2], BF16, isOutput=False)
    b1_e = nc.declare_dram_parameter("b1", [H1, 1], F32, isOutput=False)
    b2pp_e = nc.declare_dram_parameter("b2pp", [128, 1], F32, isOutput=False)
    out_e = nc.declare_dram_parameter("out", [BPC, F], F32, isOutput=True)
    dbg_e = nc.declare_dram_parameter("dbg", [H1, H1], F32, isOutput=True)
    dbg2_e = nc.declare_dram_parameter("dbg2", [128, NT + 1], F32, isOutput=True)

    with TileContext(nc) as tc:
        from contextlib import ExitStack
        with ExitStack() as _es:
            constp = _es.enter_context(tc.tile_pool(name="const", bufs=1))
            p_kn1 = _es.enter_context(tc.tile_pool(name="kn1", bufs=2))
            p_kn2 = _es.enter_context(tc.tile_pool(name="kn2", bufs=2))
            p_kT = _es.enter_context(tc.tile_pool(name="kT", bufs=4))
            p_rhs = _es.enter_context(tc.tile_pool(name="rhs", bufs=4))
            p_h1 = _es.enter_context(tc.tile_pool(name="h1sb", bufs=4))
            p_h2 = _es.enter_context(tc.tile_pool(name="h2sb", bufs=4))
            p_scores = _es.enter_context(tc.tile_pool(name="scores", bufs=2))
            p_scw = _es.enter_context(tc.tile_pool(name="scw", bufs=2))
            p_scrd = _es.enter_context(tc.tile_pool(name="scrd", bufs=2, space="DRAM"))
            p_small = _es.enter_context(tc.tile_pool(name="smalls", bufs=2))
            p_soft = _es.enter_context(tc.tile_pool(name="soft", bufs=2))
            p_pT = _es.enter_context(tc.tile_pool(name="pTp", bufs=2))
            p_outs = _es.enter_context(tc.tile_pool(name="outs", bufs=2))
            pp_h1 = _es.enter_context(tc.tile_pool(name="ph1", bufs=2, space="PSUM"))
            pp_h2 = _es.enter_context(tc.tile_pool(name="ph2", bufs=2, space="PSUM"))
            pp_sc = _es.enter_context(tc.tile_pool(name="psc", bufs=1, space="PSUM"))
            pp_small = _es.enter_context(tc.tile_pool(name="psmall", bufs=2, space="PSUM"))
            pp_out = _es.enter_context(tc.tile_pool(name="pout", bufs=1, space="PSUM"))
            ident = constp.tile([64, 64], F32)
            masks.make_identity(nc, ident[:, :])
            Ws_sb = constp.tile([128, H1], BF16)
            nc.sync.dma_start(out=Ws_sb[:, :], in_=Ws_e[:, :])
            Wqd_sb = constp.tile([F, H1], F32)
            nc.sync.dma_start(out=Wqd_sb[:, :], in_=Wqd_e[:, :])
            W2p_sb = constp.tile([H1, 64], BF16)
            nc.sync.dma_start(out=W2p_sb[:, :], in_=W2p_e[:, :])
            W3pp_sb = constp.tile([128, 2], BF16)
            nc.sync.dma_start(out=W3pp_sb[:, :], in_=W3pp_e[:, :])
            b1_sb = constp.tile([H1, 1], F32)
            nc.sync.dma_start(out=b1_sb[:, :], in_=b1_e[:, :])
            b2pp_sb = constp.tile([128, 1], F32)
            nc.sync.dma_start(out=b2pp_sb[:, :], in_=b2pp_e[:, :])
            iota_i = constp.tile([TILE, S], mybir.dt.int32)
            nc.gpsimd.iota(iota_i[:, :], pattern=[[1, S]], base=0, channel_multiplier=0)
            iota_f = constp.tile([TILE, S], F32)
            nc.vector.tensor_copy(iota_f[:, :], iota_i[:, :])
            junk_sb = constp.tile([H1, H1], F32)
            nc.vector.memset(junk_sb[:, :], 0.0)
            junk2 = constp.tile([128, NT + 1], F32)
            nc.vector.memset(junk2[:, :], 0.0)
            b1v = constp.tile([H1, 1], F32)
            nc.vector.tensor_copy(b1v[:, :], b1_sb[:, :])
            # ACT observer: introduce the b2pp DMA queue to ScalarE
            nc.scalar.activation(
                junk2[:, 0:1], b2pp_sb[:, :], mybir.ActivationFunctionType.Copy
            )

            # ---- PE semaphore observers: one fresh wait per matmul ----
            jp = pp_small.tile([H1, H1], F32, tag="ps_misc")
            nc.tensor.transpose(jp[0:64, 0:64], ident[:, :], ident[:, :])  # Pool
            nc.tensor.matmul(jp[0:H1, 0:H1], Ws_sb[:, :], Ws_sb[:, :],
                             start=True, stop=True)                        # Ws DMA q
            nc.tensor.matmul(jp[0:64, 0:64], W2p_sb[:, :], W2p_sb[:, :],
                             start=True, stop=True)                        # W2p DMA q
            nc.tensor.matmul(jp[0:2, 0:2], W3pp_sb[:, :], W3pp_sb[:, :],
                             start=True, stop=True)                        # W3pp DMA q
            nc.tensor.matmul(jp[0:H1, 0:H1], Wqd_sb[:, :], Wqd_sb[:, :],
                             start=True, stop=True)                        # Wqd DMA q
            nc.vector.tensor_copy(junk_sb[:, :], jp[:, :])

            for t in range(NT):
                b0 = t * TILE

                # ---- query (host-pretransposed) + per-batch bias A ----
                qT_sb = p_small.tile([F, TILE], F32, tag="qT")
                nc.sync.dma_start(out=qT_sb[:, :], in_=qT_e[:, b0 : b0 + TILE])
                A_ps = pp_small.tile([H1, TILE], F32, tag="ps_misc")
                nc.tensor.matmul(
                    A_ps[:, :], Wqd_sb[:, :], qT_sb[:, :], start=True, stop=True
                )
                A_sb = p_small.tile([H1, TILE], F32, tag="A")
                nc.vector.tensor_scalar_add(A_sb[:, :], A_ps[:, :], b1v[:, 0:1])
                # ACT observer: introduce this tile's A_sb (DVE tick) to ScalarE
                nc.scalar.activation(
                    junk2[0:H1, t + 1 : t + 2], A_sb[:, 0:1],
                    mybir.ActivationFunctionType.Copy,
                )

                seqt = p_small.tile([TILE, 1], F32, tag="seqt")
                nc.sync.dma_start(out=seqt[:, :], in_=seqf_e[b0 : b0 + TILE, :])

                # ---- keys: natural layout (batch-major columns), 8 batches/DMA ----
                kn1 = p_kn1.tile([128, TILE * F], BF16)
                kn2 = p_kn2.tile([80, TILE * F], BF16)
                KG = 8
                for j in range(0, TILE, KG):
                    b = b0 + j
                    nc.sync.dma_start(
                        out=kn1[:, j * F : (j + KG) * F].rearrange(
                            "p (g f) -> p g f", g=KG
                        ),
                        in_=keys_e[b : b + KG, 0:128, :].rearrange("g p f -> p g f"),
                    )
                    nc.sync.dma_start(
                        out=kn2[:, j * F : (j + KG) * F].rearrange(
                            "p (g f) -> p g f", g=KG
                        ),
                        in_=keys_e[b : b + KG, 128:S2, :].rearrange("g p f -> p g f"),
                    )
                # pair-stacked transposed keys: one DMA per tile
                kTall = p_kT.tile([128, PAIRS * S2], BF16)
                nc.sync.dma_start(
                    out=kTall[:, :].rearrange("p (g s) -> p g s", g=PAIRS),
                    in_=keysT_e[t * PAIRS : (t + 1) * PAIRS, :, :].rearrange(
                        "g p s -> p g s"
                    ),
                )

                scW = p_scw.tile([2, PAIRS * S], F32)

                # ---- per-pair: transpose, assemble rhs, MLP score ----
                for pj in range(PAIRS):
                    kT = kTall[:, pj * S2 : (pj + 1) * S2]

                    # rhs = [[kT_A; qkT_A] | [kT_B; qkT_B]]
                    rhs = p_rhs.tile([128, 2 * S2], BF16)
                    nc.vector.tensor_copy(rhs[0:F, 0:S2], kT[0:F, :])
                    if "dve2" in PROBE:
                        nc.vector.tensor_copy(rhs[0:F, 0:S2], kT[0:F, :])
                        nc.vector.tensor_copy(rhs[0:F, S2 : 2 * S2], kT[F:128, :])
                    nc.vector.tensor_scalar(
                        rhs[F:128, 0:S2], kT[0:F, :],
                        qT_sb[:, 2 * pj : 2 * pj + 1], None, mybir.AluOpType.mult,
                    )
                    nc.vector.tensor_copy(rhs[0:F, S2 : 2 * S2], kT[F:128, :])
                    nc.vector.tensor_scalar(
                        rhs[F:128, S2 : 2 * S2], kT[F:128, :],
                        qT_sb[:, 2 * pj + 1 : 2 * pj + 2], None, mybir.AluOpType.mult,
                    )
                    if "dve2" in PROBE:
                        nc.vector.tensor_scalar(
                            rhs[F:128, 0:S2], kT[0:F, :],
                            qT_sb[:, 2 * pj : 2 * pj + 1], None, mybir.AluOpType.mult,
                        )
                        nc.vector.tensor_scalar(
                            rhs[F:128, S2 : 2 * S2], kT[F:128, :],
                            qT_sb[:, 2 * pj + 1 : 2 * pj + 2], None, mybir.AluOpType.mult,
                        )

                    h1_ps = pp_h1.tile([H1, 2 * S2], F32)
                    if "pe2" in PROBE:
                        nc.tensor.matmul(
                            h1_ps[:, :], Ws_sb[:, :], rhs[:, :], start=True, stop=True
                        )
                    nc.tensor.matmul(
                        h1_ps[:, :], Ws_sb[:, :], rhs[:, :], start=True, stop=True
                    )
                    h1_sb = p_h1.tile([H1, 2 * S2], BF16)
                    if "act2" in PROBE:
                        nc.scalar.activation(
                            h1_sb[:, 0:S2], h1_ps[:, 0:S2],
                            mybir.ActivationFunctionType.Relu,
                            bias=A_sb[:, 2 * pj : 2 * pj + 1], scale=1.0,
                        )
                        nc.scalar.activation(
                            h1_sb[:, S2 : 2 * S2], h1_ps[:, S2 : 2 * S2],
                            mybir.ActivationFunctionType.Relu,
                            bias=A_sb[:, 2 * pj + 1 : 2 * pj + 2], scale=1.0,
                        )
                    if True:
                        nc.scalar.activation(
                            h1_sb[:, 0:S2], h1_ps[:, 0:S2],
                            mybir.ActivationFunctionType.Relu,
                            bias=A_sb[:, 2 * pj : 2 * pj + 1], scale=1.0,
                        )
                        nc.scalar.activation(
                            h1_sb[:, S2 : 2 * S2], h1_ps[:, S2 : 2 * S2],
                            mybir.ActivationFunctionType.Relu,
                            bias=A_sb[:, 2 * pj + 1 : 2 * pj + 2], scale=1.0,
                        )

                    h2_ps = pp_h2.tile([128, S2], F32)
                    nc.tensor.matmul(
                        h2_ps[0:64, :], W2p_sb[:, :], h1_sb[:, 0:S2],
                        start=True, stop=True, tile_position=(0, 0),
                    )
                    nc.tensor.matmul(
                        h2_ps[64:128, :], W2p_sb[:, :], h1_sb[:, S2 : 2 * S2],
                        start=True, stop=True, tile_position=(0, 64),
                    )
                    h2_sb = p_h2.tile([128, S2], BF16)
                    nc.scalar.activation(
                        h2_sb[:, :], h2_ps[:, :], mybir.ActivationFunctionType.Relu,
                        bias=b2pp_sb[:, 0:1], scale=1.0,
                    )

                    sc_ps = pp_sc.tile([2, S2], F32)
                    nc.tensor.matmul(
                        sc_ps[:, :], W3pp_sb[:, :], h2_sb[:, :], start=True, stop=True
                    )
                    nc.vector.tensor_copy(
                        scW[:, pj * S : (pj + 1) * S], sc_ps[:, 0:S]
                    )

                # regroup (parity, pair) -> batch rows through a DRAM bounce
                scr = p_scrd.tile([TILE, S], F32)
                nc.sync.dma_start(
                    out=scr[:, :].rearrange("(p two) s -> two p s", two=2),
                    in_=scW[:, :].rearrange("two (p s) -> two p s", p=PAIRS),
                )
                scores = p_scores.tile([TILE, S], F32)
                nc.sync.dma_start(out=scores[:, :], in_=scr[:, :])

                # ---- softmax over s (masked; matches reference exactly) ----
                mask = p_soft.tile([TILE, S], mybir.dt.int8, tag="mask")
                nc.vector.tensor_scalar(
                    mask[:, :], iota_f[:, :], seqt[:, 0:1], None, mybir.AluOpType.is_lt
                )
                maskd = p_soft.tile([TILE, S], F32, tag="maskd")
                nc.vector.memset(maskd[:, :], NEG_BIG)
                nc.vector.copy_predicated(maskd[:, :], mask[:, :], scores[:, :])
                rmax = p_small.tile([TILE, 1], F32, tag="rmax")
                nc.vector.tensor_reduce(
                    rmax[:, :], maskd[:, :], mybir.AxisListType.X, mybir.AluOpType.max
                )
                nrmax = p_small.tile([TILE, 1], F32, tag="nrmax")
                nc.vector.tensor_scalar_mul(nrmax[:, :], rmax[:, :], -1.0)
                ex = p_soft.tile([TILE, S], F32, tag="ex")
                rsum = p_small.tile([TILE, 1], F32, tag="rsum")
                nc.scalar.activation(
                    ex[:, :], maskd[:, :], mybir.ActivationFunctionType.Exp,
                    bias=nrmax[:, 0:1], scale=1.0, accum_out=rsum[:, 0:1],
                )
                rinv = p_small.tile([TILE, 1], F32, tag="rinv")
                nc.vector.reciprocal(rinv[:, :], rsum[:, :])
                pr = p_soft.tile([TILE, S], F32, tag="pr")
                nc.vector.tensor_scalar(
                    pr[:, :], ex[:, :], rinv[:, 0:1], None, mybir.AluOpType.mult
                )

                # ---- transpose probabilities on PE: pT[s, j] ----
                pT_ps = pp_small.tile([128, 128], F32, tag="ps_misc")
                nc.tensor.transpose(
                    pT_ps[0:128, 0:TILE], pr[:, 0:128], ident[:, :]
                )
                nc.tensor.transpose(
                    pT_ps[0:72, TILE:128], pr[:, 128:S], ident[:, :]
                )
                pT_sb = p_pT.tile([128, 128], BF16)
                nc.vector.tensor_copy(pT_sb[:, 0:TILE], pT_ps[:, 0:TILE])
                nc.vector.tensor_copy(
                    pT_sb[0:72, TILE:128], pT_ps[0:72, TILE:128]
                )

                # observer: introduce pT_sb (DVE tick) to the PE
                jp1 = pp_small.tile([H1, 2], F32, tag="ps_misc")
                nc.tensor.matmul(
                    jp1[:, 0:1], Ws_sb[:, :], pT_sb[0:128, 0:1], start=True, stop=True
                )
                nc.vector.tensor_copy(junk_sb[0:H1, 2 * t : 2 * t + 1], jp1[:, 0:1])

                # ---- output: out_b = sum_s p[b,s] * k[b,s,:] ----
                out_ps = pp_out.tile([F, TILE], F32)
                for j in range(TILE):
                    cj = j * F
                    nc.tensor.matmul(
                        out_ps[:, j : j + 1], kn1[:, cj : cj + F],
                        pT_sb[0:128, j : j + 1], start=True, stop=False,
                    )
                    nc.tensor.matmul(
                        out_ps[:, j : j + 1], kn2[0:72, cj : cj + F],
                        pT_sb[0:72, TILE + j : TILE + j + 1], start=False, stop=True,
                    )
                outT_sb = p_outs.tile([F, TILE], F32, tag="outT")
                nc.vector.tensor_copy(outT_sb[:, :], out_ps[:, :])
                outF_ps = pp_small.tile([TILE, F], F32, tag="ps_misc")
                nc.tensor.transpose(outF_ps[:, :], outT_sb[:, :], ident[:, :])
                out_sb = p_outs.tile([TILE, F], F32, tag="outf")
                nc.vector.tensor_copy(out_sb[:, :], outF_ps[:, :])
                nc.sync.dma_start(out=out_e[b0 : b0 + TILE, :], in_=out_sb[:, :])

            nc.sync.dma_start(out=dbg_e[:, :], in_=junk_sb[:, :])
            nc.sync.dma_start(out=dbg2_e[:, :], in_=junk2[:, :])

    if SPLIT_WAITS:
        _split_multi_waits(nc)
    return nc


# walrus CoreV2/V3 codegen allows only ONE sync-wait on compute instructions
# (S3_LW / S3D3 / S4D4 structs). Hoist multi-waits onto a standalone InstDrain
# (the same thing raw-bass wait_ge emits), which supports arbitrarily many.
_MULTIWAIT_OK = {
    "InstEventSemaphore",
    "InstBranch",
    "InstCompareAndBranch",
}


def _split_multi_waits(nc):
    f = nc.m.functions[0]
    n_split = 0
    for blk in f.blocks:
        insts = list(blk.instructions)
        out = []
        for inst in insts:
            tn = type(inst).__name__
            si = inst.sync_info
            waits = list(si.on_wait) if si is not None else []
            if len(waits) > 1 and tn not in _MULTIWAIT_OK:
                for w in waits:
                    d = mybir.InstDrain(
                        name=nc.get_next_instruction_name(),
                        ins=[],
                        outs=[],
                        bass_is_fusable=False,
                    )
                    d.engine = inst.engine
                    d.sync_info = mybir.SyncInfo(on_wait=[w], on_update=[])
                    out.append(d)
                inst.sync_info = mybir.SyncInfo(
                    on_wait=[], on_update=list(si.on_update)
                )
                n_split += 1
            out.append(inst)
        blk.instructions = out
    return n_split


_CACHED = {}


def _get_graph():
    if "nc" not in _CACHED:
        _CACHED["nc"] = build_graph()
    return _CACHED["nc"]


def kernel(query, keys, seq_len, W1, b1, W2, b2, W3, b3):
    query = np.asarray(query, dtype=np.float32).reshape(B, F)
    keys = np.asarray(keys, dtype=np.float32)
    seqf = np.asarray(seq_len, dtype=np.float32).reshape(B, 1)
    W1 = np.asarray(W1, dtype=np.float32)
    W2 = np.asarray(W2, dtype=np.float32)
    W3 = np.asarray(W3, dtype=np.float32)
    b1 = np.asarray(b1, dtype=np.float32)
    b2 = np.asarray(b2, dtype=np.float32)

    # weight folding (host-side constant prep)
    W1q, W1k, W1d, W1m = W1[0:F], W1[F : 2 * F], W1[2 * F : 3 * F], W1[3 * F :]
    Ws = np.concatenate([W1k - W1d, W1m], axis=0).astype(ml_dtypes.bfloat16)
    Wqd = (W1q + W1d).astype(np.float32)
    W2p = np.zeros((H1, 64), np.float32)
    W2p[:, 0:H2] = W2
    W2p = W2p.astype(ml_dtypes.bfloat16)
    W3pp = np.zeros((128, 2), np.float32)
    W3pp[0:H2, 0] = W3[:, 0]
    W3pp[64 : 64 + H2, 1] = W3[:, 0]
    W3pp = W3pp.astype(ml_dtypes.bfloat16)
    b1c = b1.reshape(H1, 1)
    b2pp = np.zeros((128, 1), np.float32)
    b2pp[0:H2, 0] = b2
    b2pp[64 : 64 + H2, 0] = b2
    # b3 is constant across s -> softmax-invariant -> dropped

    # keys: bf16, s padded 200 -> 208
    kb = np.zeros((B, S2, F), dtype=ml_dtypes.bfloat16)
    kb[:, 0:S, :] = keys.astype(ml_dtypes.bfloat16)
    # pair-stacked transposed copy: [pair, (two f), s]
    kbT = np.ascontiguousarray(
        kb.reshape(B // 2, 2, S2, F).transpose(0, 1, 3, 2).reshape(B // 2, 2 * F, S2)
    )

    nc = _get_graph()
    in_maps = []
    for i in range(NCORES):
        lo, hi = i * BPC, (i + 1) * BPC
        in_maps.append(
            {
                "keys": np.ascontiguousarray(kb[lo:hi]),
                "keysT": np.ascontiguousarray(kbT[lo // 2 : hi // 2]),
                "qT": np.ascontiguousarray(query[lo:hi].T),
                "seqf": np.ascontiguousarray(seqf[lo:hi]),
                "Ws": Ws,
                "Wqd": Wqd,
                "W2p": W2p,
                "W3pp": W3pp,
                "b1": b1c,
                "b2pp": b2pp,
            }
        )

    trace = os.environ.get("KERNEL_TRACE") == "1"
    res = run_bass_kernel_spmd(
        nc, in_maps, core_ids=list(range(NCORES)), trace=trace
    )
    _CACHED["exec_time_ns"] = getattr(res, "exec_time_ns", None)
    _CACHED["profile_json"] = getattr(res, "profile_json", None)
    out = np.concatenate([np.asarray(r["out"]) for r in res.results], axis=0)
    return out.reshape(B, 1, F).astype(np.float32)


if __name__ == "__main__":
    rng = np.random.default_rng(0)
    inputs = {
        "query": rng.standard_normal((B, 1, F), dtype=np.float32),
        "keys": rng.standard_normal((B, S, F), dtype=np.float32),
        "seq_len": rng.integers(0, S, size=(B, 1)).astype(np.int64),
        "W1": rng.standard_normal((4 * F, H1), dtype=np.float32) / 16,
        "b1": np.zeros(H1, np.float32),
        "W2": rng.standard_normal((H1, H2), dtype=np.float32) / 9,
        "b2": np.zeros(H2, np.float32),
        "W3": rng.standard_normal((H2, 1), dtype=np.float32) / 6.3,
        "b3": np.zeros(1, np.float32),
    }
    out = kernel(**inputs)
    print("out", out.shape, out.dtype)

